# revision 1
# baseline (speedup 1.0000x reference)
"""Trainium2 Bass kernel for nn_Block_17540646437178 (dense transformer block).

Sharding: data-parallel over B=16 across 8 NeuronCores (2 samples/core,
zero collectives). All matmuls run in bf16 with f32 PSUM accumulation.

Host-side folding (exact, f32): layernorm affines fold into the following
matmul weights/biases; the attention scale folds into W_q; gamma_1/gamma_2
fold into w_proj/fc2; the proj bias folds into a pre-biased residual copy
of x ("xb"); all remaining small biases ship as one packed [128, X] tile.

Attention layout: scores are computed TRANSPOSED (k-tokens on partitions)
so (a) the key-padding mask becomes a per-partition Exp bias, (b) softmax
needs no max-subtraction (logits are O(1); masked lanes underflow to 0),
(c) exp(s + rpb + mask) = exp(s + mask) * exp(rpb) with exp(rpb)
precomputed on host, making the rpb contribution a cheap bf16 multiply
split across DVE and GpSimd. V carries an appended ones-column so the
softmax denominator falls out of the attn@V matmul (column 64 of each
head's 65-wide block), landing per-partition for the normalize multiply.

The text/img FFN split (tokens 0:40 vs 40:616) is handled by DMA-repacking
the post-attention residual into [80, C] and [1152 = 9x128, C] buffers so
every FFN matmul is 128-aligned. DMA *instruction count* on the HWDGE
queues is minimized (each costs ~0.6us serially); latency-insensitive
DMAs (repack, residual reloads, output stores) run on the GpSimd SWDGE.
"""

import numpy as np
import ml_dtypes

BF16NP = ml_dtypes.bfloat16

B, N, C, H, D = 16, 616, 768, 12, 64
TXT = 40
DFF = 3072
NCORES = 8
S = B // NCORES          # 2 samples per core
EPS = 1e-5
SCALE = D ** -0.5
KC = C // 128            # 6 k-tiles over C
MQK = (2 * C) // 128     # 12 m-tiles over q+k features
KF = DFF // 128          # 24 k-tiles over dff
NT = 5                   # token tiles per sample (616 = 4*128 + 104)
TOK_TILES = [(0, 128), (128, 128), (256, 128), (384, 128), (512, 104)]
Q_CHUNKS = [(0, 512), (512, 104)]    # 616 free-dim chunks
C_CHUNKS = [(0, 512), (512, 256)]    # 768 free-dim chunks
IMG = N - TXT            # 576
IMGTOK = S * IMG         # 1152 = 9*128
TXTTOK = S * TXT         # 80
IMG_CHUNK = 384          # img token chunk for FFN (3 chunks)
NEG = -30000.0


def _slab_kp(wt):
    """[K, M] (K = KT*128) -> [128, KT, M] slab layout (partition-major)."""
    k, m = wt.shape
    kt = k // 128
    assert kt * 128 == k
    return np.ascontiguousarray(wt.reshape(kt, 128, m).transpose(1, 0, 2))


def _bf(a):
    return np.ascontiguousarray(a.astype(np.float32)).astype(BF16NP)


def _f32(a):
    return np.ascontiguousarray(np.asarray(a, dtype=np.float32))


def _bcast128(v):
    return np.ascontiguousarray(np.broadcast_to(v.astype(np.float32), (128, v.shape[0])))


def _colmajor(v, nt):
    """(nt*128,) -> [128, nt] with column t holding partitions of tile t."""
    return np.ascontiguousarray(v.astype(np.float32).reshape(nt, 128).T)


def host_prep(inputs):
    """Fold affines/scales into weights; build slab/broadcast layouts.

    Returns (shared, per_core) where per_core is a list of dicts.
    """
    inp = {k: _f32(v) if np.asarray(v).dtype != np.int32 else np.asarray(v)
           for k, v in inputs.items()}

    g1, g2 = inp["gamma_1"], inp["gamma_2"]

    # --- attention: fold ln1 affine + SCALE into w_qkv ---
    wqkv = inp["w_qkv"] * inp["ln1_g"][None, :]
    qkv_b = np.concatenate([inp["q_bias"],
                            np.zeros_like(inp["v_bias"]),
                            inp["v_bias"]])
    qkv_b = qkv_b + inp["w_qkv"] @ inp["ln1_b"]
    wqkv[:C] *= SCALE
    qkv_b[:C] *= SCALE

    w_qk = _slab_kp(_bf(wqkv[: 2 * C].T))            # [128, 6, 1536] bf16
    w_v = _slab_kp(_bf(wqkv[2 * C:].T))              # [128, 6, 768] bf16
    qk_bias = _colmajor(qkv_b[: 2 * C], MQK)         # [128, 12] f32
    v_bias = _bcast128(qkv_b[2 * C:])                # [128, 768] f32

    # --- proj: fold gamma_1 ---
    wpj = g1[:, None] * inp["w_proj"]
    w_pj = _slab_kp(_bf(wpj.T))                      # [128, 6, 768] bf16
    b_pj = _bcast128(g1 * inp["b_proj"])             # [128, 768] f32

    # --- FFN branches: fold ln2 affine into fc1, gamma_2 into fc2 ---
    def ffn(w1, b1, w2, b2, lg, lb):
        w1e = w1 * lg[None, :]
        b1e = b1 + w1 @ lb
        w2e = g2[:, None] * w2
        b2e = g2 * b2
        return w1e, b1e, w2e, b2e

    w1t, b1t, w2t, b2t = ffn(inp["fc1t_w"], inp["fc1t_b"], inp["fc2t_w"],
                             inp["fc2t_b"], inp["ln2t_g"], inp["ln2t_b"])
    w1i, b1i, w2i, b2i = ffn(inp["fc1i_w"], inp["fc1i_b"], inp["fc2i_w"],
                             inp["fc2i_b"], inp["ln2i_g"], inp["ln2i_b"])

    # text fc1 weights grouped by M-slab for streaming: [24, 128, 6, 128]
    w1t_T = _bf(w1t.T)                               # [768, 3072]
    w1t_m = np.ascontiguousarray(
        w1t_T.reshape(KC, 128, KF, 128).transpose(2, 1, 0, 3))
    w2t_k = np.ascontiguousarray(_bf(w2t.T).reshape(KF, 128, C))  # [24,128,768]
    w1i_s = _slab_kp(_bf(w1i.T))                     # [128, 6, 3072]
    w2i_s = _slab_kp(_bf(w2i.T))                     # [128, 24, 768]
    b1t_c = _colmajor(b1t, KF)                       # [128, 24]
    b1i_c = _colmajor(b1i, KF)
    b2t_b = _bcast128(b2t)                           # [128, 768]
    b2i_b = _bcast128(b2i)

    # --- exp(rpb) transposed + k-padded slabs: [12, 128, 5, 616] bf16.
    # Softmax uses exp(s + rpb + maskb) = exp(s + maskb) * exp(rpb); the
    # multiply runs in bf16 on DVE/POOL instead of an f32 PSUM add on DVE.
    rpbT = np.transpose(inp["relative_position_bias"], (0, 2, 1))  # [H, k, q]
    rpb_pad = np.zeros((H, NT * 128, N), np.float32)
    rpb_pad[:, :N, :] = np.exp(rpbT)
    rpb_slab = _bf(np.ascontiguousarray(
        rpb_pad.reshape(H, NT, 128, N).transpose(0, 2, 1, 3)))

    bias_pack = np.concatenate(
        [qk_bias, b1t_c, b1i_c, v_bias, b2t_b, b2i_b], axis=1)
    shared = dict(w_qk=w_qk, w_v=w_v, bias_pack=np.ascontiguousarray(bias_pack),
                  w_pj=w_pj, rpb=rpb_slab, w1t=w1t_m, w2t=w2t_k,
                  w1i=w1i_s, w2i=w2i_s)

    # --- per-core: x shard + mask bias ---
    mask = np.asarray(inputs["mask"]).astype(np.float32)   # [B, N] 0/1
    mb_full = (1.0 - mask) * NEG                            # [B, N]
    mb_pad = np.full((B, NT * 128), NEG, np.float32)
    mb_pad[:, :N] = mb_full
    # xb = x with the (gamma_1-folded) proj bias pre-added: the proj
    # residual eviction then needs a single tensor_add.
    xb_full = inp["x"] + (g1 * inp["b_proj"])[None, None, :]
    per_core = []
    for c in range(NCORES):
        xs = np.ascontiguousarray(inp["x"][c * S:(c + 1) * S])
        xbs = np.ascontiguousarray(xb_full[c * S:(c + 1) * S]).astype(np.float32)
        mb = np.ascontiguousarray(
            mb_pad[c * S:(c + 1) * S].reshape(S, NT, 128).transpose(0, 2, 1))
        per_core.append(dict(x=xs, xb=xbs, maskb=mb))
    return shared, per_core


def build_program(ablate=None):
    """Build the per-core Bass/Tile program. Returns compiled nc.

    ablate: None/"full", or one of "ln","qkv","attn","proj" to stop
    emission after that phase (timing experiments only — output garbage).
    """
    import os
    if ablate is None:
        ablate = os.environ.get("KERNEL_ABLATE", "full")
    LVL = {"ln": 1, "qkv": 2, "attn": 3, "proj": 4, "full": 9}[ablate]
    off = set(os.environ.get("KERNEL_OFF", "").split(","))
    # tensor_tensor_reduce is a custom DVE ISA op whose ucode is not loaded
    # on this deployment — using it hangs the device. Permanently off.
    USE_TTR = False
    POOL_MUL = "poolmul" not in off   # exp*erpb multiplies on GpSimd
    POOL_DMA = "pooldma" not in off   # late DMAs on GpSimd SWDGE
    REPS = int(os.environ.get("KERNEL_REPS", "1"))
    from contextlib import ExitStack
    import concourse.bass as bass
    import concourse.mybir as mybir
    import concourse.tile as tile
    from concourse import bacc
    from concourse.masks import make_identity

    f32 = mybir.dt.float32
    bf16 = mybir.dt.bfloat16
    Af = mybir.ActivationFunctionType
    Ax = mybir.AxisListType
    Op = mybir.AluOpType

    nc = bacc.Bacc("TRN2", target_bir_lowering=False, debug=False,
                   num_devices=NCORES)

    x_d = nc.declare_dram_parameter("x", [S, N, C], f32, isOutput=False)
    xb_d = nc.declare_dram_parameter("xb", [S, N, C], f32, isOutput=False)
    mb_d = nc.declare_dram_parameter("maskb", [S, 128, NT], f32, isOutput=False)
    wqk_d = nc.declare_dram_parameter("w_qk", [128, KC, 2 * C], bf16, isOutput=False)
    wv_d = nc.declare_dram_parameter("w_v", [128, KC, C], bf16, isOutput=False)
    bp_d = nc.declare_dram_parameter("bias_pack",
                                     [128, MQK + 2 * KF + 3 * C], f32,
                                     isOutput=False)
    wpj_d = nc.declare_dram_parameter("w_pj", [128, KC, C], bf16, isOutput=False)
    rpb_d = nc.declare_dram_parameter("rpb", [H, 128, NT, N], bf16, isOutput=False)
    w1t_d = nc.declare_dram_parameter("w1t", [KF, 128, KC, 128], bf16, isOutput=False)
    w2t_d = nc.declare_dram_parameter("w2t", [KF, 128, C], bf16, isOutput=False)
    w1i_d = nc.declare_dram_parameter("w1i", [128, KC, DFF], bf16, isOutput=False)
    w2i_d = nc.declare_dram_parameter("w2i", [128, KF, C], bf16, isOutput=False)
    out_d = nc.declare_dram_parameter("out", [S, N, C], f32, isOutput=True)

    with tile.TileContext(nc, pool_alloc_mode="queue") as tc, \
            ExitStack() as ctx:
        # ---------- pools ----------
        pers = ctx.enter_context(tc.tile_pool(name="pers", bufs=1))
        psum = ctx.enter_context(tc.tile_pool(name="psum", bufs=1, space="PSUM"))

        def ps_tile(name, wide):
            if wide > 256:
                return psum.tile([128, 512], f32, name=name, tag="big", bufs=3)
            return psum.tile([128, 256], f32, name=name, tag="sm", bufs=1)

        # ---------- persistent constants ----------
        ident = pers.tile([128, 128], bf16, name="ident")
        make_identity(nc, ident)
        bias_pack = pers.tile([128, MQK + 2 * KF + 3 * C], f32,
                              name="bias_pack")
        qkb = bias_pack[:, 0:MQK]
        b1t = bias_pack[:, MQK:MQK + KF]
        b1i = bias_pack[:, MQK + KF:MQK + 2 * KF]
        vb = bias_pack[:, MQK + 2 * KF:MQK + 2 * KF + C]
        b2t = bias_pack[:, MQK + 2 * KF + C:MQK + 2 * KF + 2 * C]
        b2i = bias_pack[:, MQK + 2 * KF + 2 * C:MQK + 2 * KF + 3 * C]
        mb = pers.tile([128, S, NT], f32, name="mb")
        x2rep_img = pers.tile([128, 9, C], f32, name="x2rep_img")
        x2rep_txt = pers.tile([128, C], f32, name="x2rep_txt")
        eps_t = pers.tile([128, 1], f32, name="eps_t")
        nc.vector.memset(eps_t[:], EPS)

        # ---------- helpers ----------
        def layer_norm(pool, src_ap, tp, dst_ap):
            """dst(bf16) = (src - mean)/sqrt(var+EPS); src [tp, C] f32."""
            sm = pool.tile([128, 1], f32, name="ln_sm", tag="ln_sm", bufs=4)
            nc.vector.tensor_reduce(sm[0:tp], src_ap, Ax.X, Op.add)
            nm = pool.tile([128, 1], f32, name="ln_nm", tag="ln_nm", bufs=4)
            nc.scalar.mul(nm[0:tp], sm[0:tp], -1.0 / C)
            xc = pool.tile([128, C], f32, name="ln_xc", tag="ln_xc", bufs=2)
            nc.vector.tensor_scalar_add(xc[0:tp], src_ap, nm[0:tp])
            sq = pool.tile([128, C], f32, name="ln_sq", tag="ln_sq", bufs=2)
            ssq = pool.tile([128, 1], f32, name="ln_ssq", tag="ln_ssq", bufs=4)
            if USE_TTR:
                nc.vector.tensor_tensor_reduce(
                    sq[0:tp], xc[0:tp], xc[0:tp], 1.0, 0.0,
                    Op.mult, Op.add, ssq[0:tp])
            else:
                nc.scalar.activation(sq[0:tp], xc[0:tp], Af.Square,
                                     accum_out=ssq[0:tp])
            std = pool.tile([128, 1], f32, name="ln_std", tag="ln_std", bufs=4)
            nc.scalar.activation(std[0:tp], ssq[0:tp], Af.Sqrt,
                                 bias=eps_t[0:tp], scale=1.0 / C)
            rstd = pool.tile([128, 1], f32, name="ln_rstd", tag="ln_rstd", bufs=4)
            nc.vector.reciprocal(rstd[0:tp], std[0:tp])
            nc.vector.tensor_scalar_mul(dst_ap, xc[0:tp], rstd[0:tp])

        def late_dma(out_ap, in_ap):
            (nc.gpsimd if POOL_DMA else nc.sync).dma_start(out_ap, in_ap)

        tp_flip = [0]

        def transpose_block(src_full_ap, dst_full_ap):
            """dst[128,128] = src[128,128].T via PE; evictions alternate
            ACT/DVE to balance engine load. Rows beyond the valid token
            count carry garbage into padded dst columns (never read)."""
            ps = psum.tile([128, 128], bf16, name="tps", tag="tp", bufs=2)
            nc.tensor.transpose(ps[:], src_full_ap, ident[:])
            tp_flip[0] ^= 1
            if tp_flip[0]:
                nc.scalar.copy(dst_full_ap, ps[:])
            else:
                nc.vector.tensor_copy(dst_full_ap, ps[:])

        for _rep in range(REPS):
            # ================= attention era =================
            with tc.tile_pool(name="era", bufs=1) as era:
                xT = {}
                qkT = {}
                vsb = {}
                osb = {}
                x2 = {}

                with tc.tile_pool(name="wqkv", bufs=1) as wp:
                    wqk = wp.tile([128, KC, 2 * C], bf16, name="wqk")
                    wv = wp.tile([128, KC, C], bf16, name="wv")

                    # ---- LN1 + transpose to xT ----
                    for s in range(S):
                        xT[s] = era.tile([128, KC, 640], bf16, name=f"xT{s}",
                                         tag="xT", bufs=2)
                        for (t0, tp) in TOK_TILES:
                            xin = era.tile([128, C], f32, name="xin", tag="xin",
                                           bufs=4)
                            nc.sync.dma_start(xin[0:tp], x_d[s, t0:t0 + tp, :])
                            xh = era.tile([128, C], bf16, name="xh", tag="xh",
                                          bufs=3)
                            if tp < 128:
                                nc.vector.memset(xh[96:128, :], 0.0)
                            layer_norm(era, xin[0:tp], tp, xh[0:tp])
                            for f in range(KC):
                                transpose_block(xh[:, f * 128:(f + 1) * 128],
                                                xT[s][:, f, t0:t0 + 128])

                    if _rep == 0:
                        nc.sync.dma_start(bias_pack[:], bp_d[:])
                        nc.sync.dma_start(mb[:],
                                          mb_d[:].rearrange("s p t -> p s t"))
                    nc.sync.dma_start(wqk[:], wqk_d[:])
                    nc.sync.dma_start(wv[:], wv_d[:])
                    # ---- QKV projections ----
                    for s in range(S if LVL >= 2 else 0):
                        qkT[s] = era.tile([128, MQK, N], bf16, name=f"qkT{s}",
                                          tag="qkT", bufs=2)
                        for m in range(MQK):
                            for (q0, qn) in Q_CHUNKS:
                                ps = ps_tile("ps_qk", qn)
                                for k in range(KC):
                                    nc.tensor.matmul(
                                        ps[:, 0:qn],
                                        wqk[:, k, m * 128:(m + 1) * 128],
                                        xT[s][:, k, q0:q0 + qn],
                                        start=(k == 0), stop=(k == KC - 1))
                                nc.vector.tensor_scalar_add(
                                    qkT[s][:, m, q0:q0 + qn], ps[:, 0:qn],
                                    qkb[:, m:m + 1])
                        vsb[s] = era.tile([128, NT, H * 65], bf16, name=f"v{s}",
                                          tag="v", bufs=2)
                        for ti, (t0, tp) in enumerate(TOK_TILES):
                            for (n0, nn) in C_CHUNKS:
                                ps = ps_tile("ps_v", nn)
                                for k in range(KC):
                                    nc.tensor.matmul(
                                        ps[0:tp, 0:nn],
                                        xT[s][:, k, t0:t0 + tp],
                                        wv[:, k, n0:n0 + nn],
                                        start=(k == 0), stop=(k == KC - 1))
                                nh = nn // 64
                                h0 = n0 // 64
                                vview = vsb[s][0:tp, ti, :].rearrange(
                                    "p (h e) -> p h e", e=65)[:, h0:h0 + nh, 0:64]
                                nc.vector.tensor_add(
                                    vview,
                                    ps[0:tp, 0:nn].rearrange("p (h e) -> p h e",
                                                             e=64),
                                    vb[0:tp, n0:n0 + nn].rearrange(
                                        "p (h e) -> p h e", e=64))
                            ones = vsb[s][0:tp, ti, :].rearrange(
                                "p (h e) -> p h e", e=65)[:, :, 64:65]
                            nc.vector.memset(ones, 1.0)

                # ---- attention core ----
                for s in range(S if LVL >= 3 else 0):
                    osb[s] = era.tile([128, NT, C], bf16, name=f"o{s}",
                                      tag="o", bufs=2)
                    nc.vector.memset(osb[s][96:128, NT - 1, :], 0.0)
                with tc.tile_pool(name="attn", bufs=1) as apool:
                    for s in range(S if LVL >= 3 else 0):
                        for h in range(H):
                            rpb = apool.tile([128, NT, N], bf16, name="rpb",
                                             tag="rpb", bufs=2)
                            nc.sync.dma_start(rpb[:], rpb_d[h])
                            mtile = KC + h // 2
                            qtile = h // 2
                            base = (h % 2) * 64
                            expT = apool.tile([128, NT, N], bf16, name="expT",
                                              tag="expT", bufs=2)
                            for kt, (k0, tp) in enumerate(TOK_TILES):
                                eraw = apool.tile([128, N], bf16, name="eraw",
                                                  tag="eraw", bufs=4)
                                for (q0, qn) in Q_CHUNKS:
                                    ps = ps_tile("ps_sc", qn)
                                    nc.tensor.matmul(
                                        ps[0:tp, 0:qn],
                                        qkT[s][base:base + 64, mtile, k0:k0 + tp],
                                        qkT[s][base:base + 64, qtile, q0:q0 + qn],
                                        start=True, stop=True)
                                    nc.scalar.activation(
                                        eraw[0:tp, q0:q0 + qn],
                                        ps[0:tp, 0:qn], Af.Exp,
                                        bias=mb[0:tp, s, kt:kt + 1])
                                eng = (nc.gpsimd if (POOL_MUL and kt % 3 == 2)
                                       else nc.vector)
                                eng.tensor_mul(expT[0:tp, kt, :],
                                               eraw[0:tp, :],
                                               rpb[0:tp, kt, :])
                            for qt, (qq0, qp) in enumerate(TOK_TILES):
                                ops = psum.tile([128, 65], f32, name="ops",
                                                tag="tiny", bufs=2)
                                for kt, (k0, tp) in enumerate(TOK_TILES):
                                    nc.tensor.matmul(
                                        ops[0:qp, :],
                                        expT[0:tp, kt, qq0:qq0 + qp],
                                        vsb[s][0:tp, kt, h * 65:(h + 1) * 65],
                                        start=(kt == 0), stop=(kt == NT - 1))
                                rc = era.tile([128, 1], f32, name="rc", tag="rc",
                                              bufs=4)
                                nc.vector.reciprocal(rc[0:qp], ops[0:qp, 64:65])
                                nc.vector.tensor_scalar_mul(
                                    osb[s][0:qp, qt, h * 64:(h + 1) * 64],
                                    ops[0:qp, 0:64], rc[0:qp])

                # ---- proj + residual ----
                with tc.tile_pool(name="proj", bufs=1) as pp:
                    wpj = pp.tile([128, KC, C], bf16, name="wpj")
                    nc.sync.dma_start(wpj[:], wpj_d[:])
                    for s in range(S if LVL >= 4 else 0):
                        oT = era.tile([128, KC, 640], bf16, name=f"oT{s}",
                                      tag="xT", bufs=2)
                        for ti, (t0, tp) in enumerate(TOK_TILES):
                            for f in range(KC):
                                transpose_block(
                                    osb[s][:, ti, f * 128:(f + 1) * 128],
                                    oT[:, f, t0:t0 + 128])
                        x2[s] = era.tile([128, NT, C], f32, name=f"x2_{s}",
                                         tag="x2", bufs=2)
                        for ti, (t0, tp) in enumerate(TOK_TILES):
                            xres = pp.tile([128, C], f32, name="xres", tag="xres",
                                           bufs=2)
                            late_dma(xres[0:tp], xb_d[s, t0:t0 + tp, :])
                            for (n0, nn) in C_CHUNKS:
                                ps = ps_tile("ps_pj", nn)
                                for k in range(KC):
                                    nc.tensor.matmul(
                                        ps[0:tp, 0:nn],
                                        oT[:, k, t0:t0 + tp],
                                        wpj[:, k, n0:n0 + nn],
                                        start=(k == 0), stop=(k == KC - 1))
                                nc.vector.tensor_add(
                                    x2[s][0:tp, ti, n0:n0 + nn],
                                    ps[0:tp, 0:nn], xres[0:tp, n0:n0 + nn])

                # ---- repack x2 -> text [80, C] + img [1152 (9x128), C] ----
                for s in range(S if LVL >= 4 else 0):
                    nc.sync.dma_start(x2rep_txt[40 * s:40 * s + 40, :],
                                      x2[s][0:40, 0, :])
                    # img rows: seq 40..616 -> global 576*s ..
                    g = 576 * s
                    for kt, (t0, tp) in enumerate(TOK_TILES):
                        p0 = 40 if kt == 0 else 0
                        length = tp - p0
                        src_off = p0
                        while length > 0:
                            j, dp = g // 128, g % 128
                            piece = min(length, 128 - dp)
                            nc.sync.dma_start(
                                x2rep_img[dp:dp + piece, j, :],
                                x2[s][src_off:src_off + piece, kt, :])
                            g += piece
                            src_off += piece
                            length -= piece

            # ================= FFN era =================
            if LVL >= 5:
                with tc.tile_pool(name="ffn", bufs=1) as fp:
                    w1i = fp.tile([128, KC, DFF], bf16, name="w1i")
                    w2i = fp.tile([128, KF, C], bf16, name="w2i")
                    for k in range(KC):
                        nc.sync.dma_start(w1i[:, k, :], w1i_d[:, k, :])
                    nc.sync.dma_start(w2i[:, 0:12, :], w2i_d[:, 0:12, :])
                    nc.sync.dma_start(w2i[:, 12:24, :], w2i_d[:, 12:24, :])
                    # LN2 + transpose
                    ztT = fp.tile([128, KC, 128], bf16, name="ztT")
                    xh2 = fp.tile([128, C], bf16, name="xh2", tag="xh2", bufs=2)
                    nc.vector.memset(xh2[64:128, :], 0.0)
                    layer_norm(fp, x2rep_txt[0:TXTTOK], TXTTOK, xh2[0:TXTTOK])
                    for f in range(KC):
                        transpose_block(xh2[:, f * 128:(f + 1) * 128],
                                        ztT[:, f, 0:128])
                    ziT = fp.tile([128, KC, IMGTOK], bf16, name="ziT")
                    for j in range(9):
                        xh2 = fp.tile([128, C], bf16, name="xh2", tag="xh2", bufs=2)
                        layer_norm(fp, x2rep_img[:, j, :], 128, xh2[:])
                        for f in range(KC):
                            transpose_block(xh2[:, f * 128:(f + 1) * 128],
                                            ziT[:, f, j * 128:(j + 1) * 128])
                    # Pre-add the (gamma_2-folded) fc2 biases into the residual so
                    # each fc2 eviction is a single tensor_add. In-place; Tile
                    # orders these after the LN2 reads above.
                    nc.vector.tensor_add(x2rep_txt[0:TXTTOK, :], x2rep_txt[0:TXTTOK, :],
                                         b2t[0:TXTTOK, :])
                    for j in range(9):
                        nc.vector.tensor_add(x2rep_img[:, j, :], x2rep_img[:, j, :],
                                             b2i[:, :])

                    # ---- img FFN (resident weights, 3 token chunks) ----
                    for c in range(3):
                        q0 = c * IMG_CHUNK
                        hgi = fp.tile([128, KF, IMG_CHUNK], bf16, name="hgi",
                                      tag="hgi", bufs=1)
                        for m in range(KF):
                            ps = ps_tile("ps_f1i", 512)
                            for k in range(KC):
                                nc.tensor.matmul(ps[:, 0:IMG_CHUNK],
                                                 w1i[:, k, m * 128:(m + 1) * 128],
                                                 ziT[:, k, q0:q0 + IMG_CHUNK],
                                                 start=(k == 0), stop=(k == KC - 1))
                            nc.scalar.activation(hgi[:, m, :], ps[:, 0:IMG_CHUNK],
                                                 Af.Gelu, bias=b1i[:, m:m + 1])
                        for mt in range(3):
                            j = 3 * c + mt
                            ps0 = ps_tile("ps_f2i0", 512)
                            ps1 = ps_tile("ps_f2i1", 256)
                            for k in range(KF):
                                nc.tensor.matmul(ps0[:, 0:512],
                                                 hgi[:, k, mt * 128:(mt + 1) * 128],
                                                 w2i[:, k, 0:512],
                                                 start=(k == 0), stop=(k == KF - 1))
                                nc.tensor.matmul(ps1[:, 0:256],
                                                 hgi[:, k, mt * 128:(mt + 1) * 128],
                                                 w2i[:, k, 512:768],
                                                 start=(k == 0), stop=(k == KF - 1))
                            ot = fp.tile([128, C], f32, name="ot", tag="ost", bufs=3)
                            for (n0, nn), ps in zip(C_CHUNKS, [ps0, ps1]):
                                nc.vector.tensor_add(ot[:, n0:n0 + nn], ps[:, 0:nn],
                                                     x2rep_img[:, j, n0:n0 + nn])
                            # DMA out: global img row g = 128*j -> (b, 40 + g%576)
                            g0 = 128 * j
                            p = 0
                            while p < 128:
                                g = g0 + p
                                b = g // IMG
                                piece = min(128 - p, IMG * (b + 1) - g)
                                late_dma(
                                    out_d[b, TXT + g - b * IMG:
                                          TXT + g - b * IMG + piece, :],
                                    ot[p:p + piece, :])
                                p += piece

                    # ---- text FFN (streamed weights) ----
                    with tc.tile_pool(name="wtxt", bufs=1) as wt:
                        hgt = fp.tile([128, KF, TXTTOK], bf16, name="hgt")
                        for mc in range(8):
                            w1tc = wt.tile([128, 3, KC * 128], bf16,
                                           name="w1tc", tag="w1tc", bufs=2)
                            nc.sync.dma_start(
                                w1tc[:],
                                w1t_d[3 * mc:3 * mc + 3].rearrange(
                                    "m p k n -> p m (k n)"))
                            for ml in range(3):
                                m = 3 * mc + ml
                                ps = ps_tile("ps_f1t", 512)
                                for k in range(KC):
                                    nc.tensor.matmul(
                                        ps[:, 0:TXTTOK],
                                        w1tc[:, ml, k * 128:(k + 1) * 128],
                                        ztT[:, k, 0:TXTTOK],
                                        start=(k == 0), stop=(k == KC - 1))
                                nc.scalar.activation(
                                    hgt[:, m, 0:TXTTOK], ps[:, 0:TXTTOK],
                                    Af.Gelu, bias=b1t[:, m:m + 1])
                        ps0 = ps_tile("ps_f2t0", 512)
                        ps1 = ps_tile("ps_f2t1", 256)
                        for kc4 in range(8):
                            w2tc = wt.tile([128, 3, C], bf16, name="w2tc",
                                           tag="w2tc", bufs=2)
                            nc.sync.dma_start(
                                w2tc[:],
                                w2t_d[3 * kc4:3 * kc4 + 3].rearrange(
                                    "k p n -> p k n"))
                            for kl in range(3):
                                k = 3 * kc4 + kl
                                nc.tensor.matmul(
                                    ps0[0:TXTTOK, 0:512], hgt[:, k, 0:TXTTOK],
                                    w2tc[:, kl, 0:512],
                                    start=(k == 0), stop=(k == KF - 1))
                                nc.tensor.matmul(
                                    ps1[0:TXTTOK, 0:256], hgt[:, k, 0:TXTTOK],
                                    w2tc[:, kl, 512:768],
                                    start=(k == 0), stop=(k == KF - 1))
                        ot = fp.tile([128, C], f32, name="ot", tag="ost", bufs=3)
                        for (n0, nn), ps in zip(C_CHUNKS, [ps0, ps1]):
                            nc.vector.tensor_add(ot[0:TXTTOK, n0:n0 + nn],
                                                 ps[0:TXTTOK, 0:nn],
                                                 x2rep_txt[0:TXTTOK, n0:n0 + nn])
                        for s in range(S):
                            late_dma(out_d[s, 0:TXT, :],
                                     ot[40 * s:40 * s + 40, :])

    nc.compile()
    return nc


_CACHE = {}


def _get_program():
    if "nc" not in _CACHE:
        _CACHE["nc"] = build_program()
    return _CACHE["nc"]


def run(inputs, trace=False):
    from concourse.bass_utils import run_bass_kernel_spmd
    shared, per_core = host_prep(inputs)
    nc = _get_program()
    in_maps = [{**shared, **pc} for pc in per_core]
    res = run_bass_kernel_spmd(nc, in_maps, core_ids=list(range(NCORES)),
                               trace=trace)
    out = np.concatenate([res.results[i]["out"] for i in range(NCORES)],
                         axis=0).astype(np.float32)
    return out, res


def kernel(**inputs):
    out, _ = run(inputs, trace=False)
    return out



# revision 12
# speedup vs baseline: 1.8048x; 1.8048x over previous
"""Trainium2 Bass kernel for nn_Block_17540646437178 (dense transformer block).

Sharding: data-parallel over B=16 across 8 NeuronCores (2 samples/core,
zero collectives). Big matmuls run in fp8e4 (TRN E4M3, max 240) with
DoubleRow perf mode (2 fp8 K-slabs per pass, f32 PSUM accumulation);
weights carry power-of-2 scales chosen so |w|*S < 224, descaled at PSUM
eviction where the descale folds into ops that exist anyway
(tensor_scalar / scalar_tensor_tensor / the Gelu activation's scale).
Attention scores + attn@V stay bf16 (K=64 / free-dim-65 shapes don't
benefit from DoubleRow); score matmuls for a head PAIR are issued
back-to-back on complementary PE row-halves (rows 0:63 / 64:127,
tile_position row tiling) so they run concurrently in the array.

Host-side folding (exact, f32): layernorm affines fold into the following
matmul weights/biases; the attention scale folds into W_q; gamma_1/gamma_2
fold into w_proj/fc2; all small biases ship as one packed [128, X] tile.
x is loaded once per rep and kept resident in f32; the (gamma_1-folded)
proj bias is added in place after LN1 consumes each tile, so the proj
eviction is one fused (ps*ds + xres) op and the old duplicate "xb" DMA
stream is gone.

Attention layout: scores are computed TRANSPOSED (k-tokens on partitions)
so (a) the key-padding mask becomes a per-partition Exp bias, (b) softmax
needs no max-subtraction (logits are O(1); masked lanes underflow to 0),
(c) exp(s + rpb + mask) = exp(s + mask) * exp(rpb) with exp(rpb)
precomputed on host, making the rpb contribution a cheap bf16 multiply
split across DVE and GpSimd. V carries an appended ones-column so the
softmax denominator falls out of the attn@V matmul, landing per-partition
for the normalize multiply.

The text/img FFN split (tokens 0:40 vs 40:616) is handled by DMA-repacking
the post-attention residual into [80, C] and [1152 = 9x128, C] buffers so
every FFN matmul is 128-aligned. Latency-insensitive DMAs run on the
GpSimd SWDGE.
"""

import numpy as np
import ml_dtypes

BF16NP = ml_dtypes.bfloat16
F8NP = ml_dtypes.float8_e4m3   # TRN FP8_EXP4: max +-240

B, N, C, H, D = 16, 616, 768, 12, 64
TXT = 40
DFF = 3072
NCORES = 8
S = B // NCORES          # 2 samples per core
EPS = 1e-5
SCALE = D ** -0.5
KC = C // 128            # 6 k-tiles over C
MQK = (2 * C) // 128     # 12 m-tiles over q+k features
KF = DFF // 128          # 24 k-tiles over dff
NT = 5                   # token tiles per sample (616 = 4*128 + 104)
TOK_TILES = [(0, 128), (128, 128), (256, 128), (384, 128), (512, 104)]
Q_CHUNKS = [(0, 512), (512, 104)]    # 616 free-dim chunks
C_CHUNKS = [(0, 512), (512, 256)]    # 768 free-dim chunks
IMG = N - TXT            # 576
IMGTOK = S * IMG         # 1152 = 9*128
TXTTOK = S * TXT         # 80
IMG_CHUNK = 384          # img token chunk for FFN (3 chunks)
NEG = -30000.0

# fp8 weight scales (power-of-2; chosen so scaled absmax ~150-225 < 240)
SQK = 2.0 ** 11
SV = 2.0 ** 11
SPJ = 2.0 ** 14
S1T = 2.0 ** 11
S2T = 2.0 ** 14
S1I = 2.0 ** 11
S2I = 2.0 ** 14


def _slab_kp(wt):
    """[K, M] (K = KT*128) -> [128, KT, M] slab layout (partition-major)."""
    k, m = wt.shape
    kt = k // 128
    assert kt * 128 == k
    return np.ascontiguousarray(wt.reshape(kt, 128, m).transpose(1, 0, 2))


def _bf(a):
    return np.ascontiguousarray(a.astype(np.float32)).astype(BF16NP)


def _f8(a, scale):
    x = np.ascontiguousarray(a.astype(np.float32)) * scale
    return np.clip(x, -224.0, 224.0).astype(F8NP)


def _f32(a):
    return np.ascontiguousarray(np.asarray(a, dtype=np.float32))


def _bcast128(v):
    return np.ascontiguousarray(np.broadcast_to(v.astype(np.float32), (128, v.shape[0])))


def _colmajor(v, nt):
    """(nt*128,) -> [128, nt] with column t holding partitions of tile t."""
    return np.ascontiguousarray(v.astype(np.float32).reshape(nt, 128).T)


def host_prep(inputs):
    """Fold affines/scales into weights; build slab/broadcast layouts.

    Returns (shared, per_core) where per_core is a list of dicts.
    """
    inp = {k: _f32(v) if np.asarray(v).dtype != np.int32 else np.asarray(v)
           for k, v in inputs.items()}

    g1, g2 = inp["gamma_1"], inp["gamma_2"]

    # --- attention: fold ln1 affine + SCALE into w_qkv ---
    wqkv = inp["w_qkv"] * inp["ln1_g"][None, :]
    qkv_b = np.concatenate([inp["q_bias"],
                            np.zeros_like(inp["v_bias"]),
                            inp["v_bias"]])
    qkv_b = qkv_b + inp["w_qkv"] @ inp["ln1_b"]
    # NOTE: the attention SCALE is NOT folded into W_q (it would push q into
    # fp8's subnormal range); it rides the Exp activation's scale operand.

    w_qk = _slab_kp(_f8(wqkv[: 2 * C].T, SQK))       # [128, 6, 1536] f8
    w_v = _slab_kp(_f8(wqkv[2 * C:].T, SV))          # [128, 6, 768] f8
    # qk bias ships PRE-SCALED: eviction computes (ps + S*b) * (1/S)
    qk_bias = _colmajor(qkv_b[: 2 * C] * SQK, MQK)   # [128, 12] f32
    v_bias = _bcast128(qkv_b[2 * C:])                # [128, 768] f32 (true)

    # --- proj: fold gamma_1 ---
    wpj = g1[:, None] * inp["w_proj"]
    w_pj = _slab_kp(_f8(wpj.T, SPJ))                 # [128, 6, 768] f8
    b_pj = _bcast128(g1 * inp["b_proj"])             # [128, 768] f32

    # --- FFN branches: fold ln2 affine into fc1, gamma_2 into fc2 ---
    def ffn(w1, b1, w2, b2, lg, lb):
        w1e = w1 * lg[None, :]
        b1e = b1 + w1 @ lb
        w2e = g2[:, None] * w2
        b2e = g2 * b2
        return w1e, b1e, w2e, b2e

    w1t, b1t, w2t, b2t = ffn(inp["fc1t_w"], inp["fc1t_b"], inp["fc2t_w"],
                             inp["fc2t_b"], inp["ln2t_g"], inp["ln2t_b"])
    w1i, b1i, w2i, b2i = ffn(inp["fc1i_w"], inp["fc1i_b"], inp["fc2i_w"],
                             inp["fc2i_b"], inp["ln2i_g"], inp["ln2i_b"])

    # text fc1 weights grouped by M-slab for streaming: [24, 128, 6, 128] f8
    w1t_T = _f8(w1t.T, S1T)                          # [768, 3072]
    w1t_m = np.ascontiguousarray(
        w1t_T.reshape(KC, 128, KF, 128).transpose(2, 1, 0, 3))
    w2t_k = np.ascontiguousarray(
        _f8(w2t.T, S2T).reshape(KF, 128, C))         # [24, 128, 768] f8
    w1i_s = _slab_kp(_f8(w1i.T, S1I))                # [128, 6, 3072] f8
    w2i_s = _slab_kp(_f8(w2i.T, S2I))                # [128, 24, 768] f8
    b1t_c = _colmajor(b1t, KF)                       # [128, 24] f32 (true)
    b1i_c = _colmajor(b1i, KF)
    b2t_b = _bcast128(b2t)                           # [128, 768] f32 (true)
    b2i_b = _bcast128(b2i)

    # --- exp(rpb) transposed + k-padded slabs: [12, 128, 5, 616] fp8.
    # Softmax uses exp(s + rpb + maskb) = exp(s + maskb) * exp(rpb); the
    # multiply runs on DVE/POOL instead of an f32 PSUM add on DVE. exp(rpb)
    # is in [0.5, 2.1] so fp8e4 (~4% rel err on attention weights, averaged
    # away by softmax) is fine and halves both SBUF and HBM traffic.
    rpbT = np.transpose(inp["relative_position_bias"], (0, 2, 1))  # [H, k, q]
    rpb_pad = np.zeros((H, NT * 128, N), np.float32)
    rpb_pad[:, :N, :] = np.exp(rpbT)
    rpb_slab = _f8(np.ascontiguousarray(
        rpb_pad.reshape(H, NT, 128, N).transpose(0, 2, 1, 3)), 1.0)

    bias_pack = np.concatenate(
        [qk_bias, b1t_c, b1i_c, v_bias, b2t_b, b2i_b, b_pj], axis=1)
    shared = dict(w_qk=w_qk, w_v=w_v, bias_pack=np.ascontiguousarray(bias_pack),
                  w_pj=w_pj, rpb=rpb_slab, w1t=w1t_m, w2t=w2t_k,
                  w1i=w1i_s, w2i=w2i_s)

    # --- per-core: x shard + mask bias ---
    mask = np.asarray(inputs["mask"]).astype(np.float32)   # [B, N] 0/1
    mb_full = (1.0 - mask) * NEG                            # [B, N]
    mb_pad = np.full((B, NT * 128), NEG, np.float32)
    mb_pad[:, :N] = mb_full
    per_core = []
    for c in range(NCORES):
        xs = np.ascontiguousarray(inp["x"][c * S:(c + 1) * S])
        mb = np.ascontiguousarray(
            mb_pad[c * S:(c + 1) * S].reshape(S, NT, 128).transpose(0, 2, 1))
        per_core.append(dict(x=xs, maskb=mb))
    return shared, per_core


def build_program(ablate=None):
    """Build the per-core Bass/Tile program. Returns compiled nc.

    ablate: None/"full", or one of "ln","qkv","attn","proj" to stop
    emission after that phase (timing experiments only — output garbage).
    """
    import os
    if ablate is None:
        ablate = os.environ.get("KERNEL_ABLATE", "full")
    LVL = {"ln": 1, "qkv": 2, "attn": 3, "proj": 4, "full": 9}[ablate]
    off = set(os.environ.get("KERNEL_OFF", "").split(","))
    POOL_MUL = "poolmul" not in off   # exp*erpb multiplies on GpSimd
    POOL_DMA = "pooldma" not in off   # late DMAs on GpSimd SWDGE
    REPS = int(os.environ.get("KERNEL_REPS", "1"))
    from contextlib import ExitStack
    import concourse.bass as bass
    import concourse.mybir as mybir
    import concourse.tile as tile
    from concourse import bacc
    from concourse.masks import make_identity

    f32 = mybir.dt.float32
    bf16 = mybir.dt.bfloat16
    f8 = mybir.dt.float8e4
    Af = mybir.ActivationFunctionType
    Ax = mybir.AxisListType
    Op = mybir.AluOpType
    DR = mybir.MatmulPerfMode.DoubleRow

    DS_QK = 1.0 / SQK
    DS_V = 1.0 / SV
    DS_PJ = 1.0 / SPJ
    DS_1T = 1.0 / S1T
    DS_2T = 1.0 / S2T
    DS_1I = 1.0 / S1I
    DS_2I = 1.0 / S2I

    nc = bacc.Bacc("TRN2", target_bir_lowering=False, debug=False,
                   num_devices=NCORES)

    x_d = nc.declare_dram_parameter("x", [S, N, C], f32, isOutput=False)
    mb_d = nc.declare_dram_parameter("maskb", [S, 128, NT], f32, isOutput=False)
    wqk_d = nc.declare_dram_parameter("w_qk", [128, KC, 2 * C], f8, isOutput=False)
    wv_d = nc.declare_dram_parameter("w_v", [128, KC, C], f8, isOutput=False)
    bp_d = nc.declare_dram_parameter("bias_pack",
                                     [128, MQK + 2 * KF + 4 * C], f32,
                                     isOutput=False)
    wpj_d = nc.declare_dram_parameter("w_pj", [128, KC, C], f8, isOutput=False)
    rpb_d = nc.declare_dram_parameter("rpb", [H, 128, NT, N], f8, isOutput=False)
    w1t_d = nc.declare_dram_parameter("w1t", [KF, 128, KC, 128], f8, isOutput=False)
    w2t_d = nc.declare_dram_parameter("w2t", [KF, 128, C], f8, isOutput=False)
    w1i_d = nc.declare_dram_parameter("w1i", [128, KC, DFF], f8, isOutput=False)
    w2i_d = nc.declare_dram_parameter("w2i", [128, KF, C], f8, isOutput=False)
    out_d = nc.declare_dram_parameter("out", [S, N, C], f32, isOutput=True)

    with tile.TileContext(nc, pool_alloc_mode="queue") as tc, \
            ExitStack() as ctx:
        # ---------- pools ----------
        pers = ctx.enter_context(tc.tile_pool(name="pers", bufs=1))
        psum = ctx.enter_context(tc.tile_pool(name="psum", bufs=1, space="PSUM"))

        def ps_tile(name, wide, force_big=False):
            if wide > 256 or force_big:
                return psum.tile([128, 512], f32, name=name, tag="big", bufs=3)
            return psum.tile([128, 256], f32, name=name, tag="sm", bufs=1)

        # ---------- persistent constants ----------
        ident = pers.tile([128, 128], bf16, name="ident")
        make_identity(nc, ident)
        bias_pack = pers.tile([128, MQK + 2 * KF + 4 * C], f32,
                              name="bias_pack")
        qkb = bias_pack[:, 0:MQK]
        b1t = bias_pack[:, MQK:MQK + KF]
        b1i = bias_pack[:, MQK + KF:MQK + 2 * KF]
        _o = MQK + 2 * KF
        vb = bias_pack[:, _o:_o + C]
        b2t = bias_pack[:, _o + C:_o + 2 * C]
        b2i = bias_pack[:, _o + 2 * C:_o + 3 * C]
        bpj = bias_pack[:, _o + 3 * C:_o + 4 * C]
        mb = pers.tile([128, S, NT], f32, name="mb")
        x2rep_img = pers.tile([128, 9, C], f32, name="x2rep_img")
        x2rep_txt = pers.tile([128, C], f32, name="x2rep_txt")
        eps_t = pers.tile([128, 1], f32, name="eps_t")
        nc.vector.memset(eps_t[:], EPS)

        # ---------- helpers ----------
        def layer_norm(pool, src_ap, tp, dst_ap):
            """dst(bf16) = (src - mean)/sqrt(var+EPS); src [tp, C] f32."""
            sm = pool.tile([128, 1], f32, name="ln_sm", tag="ln_sm", bufs=4)
            nc.vector.tensor_reduce(sm[0:tp], src_ap, Ax.X, Op.add)
            nm = pool.tile([128, 1], f32, name="ln_nm", tag="ln_nm", bufs=4)
            nc.scalar.mul(nm[0:tp], sm[0:tp], -1.0 / C)
            xc = pool.tile([128, C], f32, name="ln_xc", tag="ln_xc", bufs=2)
            nc.vector.tensor_scalar_add(xc[0:tp], src_ap, nm[0:tp])
            sq = pool.tile([128, C], f32, name="ln_sq", tag="ln_sq", bufs=2)
            ssq = pool.tile([128, 1], f32, name="ln_ssq", tag="ln_ssq", bufs=4)
            nc.scalar.activation(sq[0:tp], xc[0:tp], Af.Square,
                                 accum_out=ssq[0:tp])
            std = pool.tile([128, 1], f32, name="ln_std", tag="ln_std", bufs=4)
            nc.scalar.activation(std[0:tp], ssq[0:tp], Af.Sqrt,
                                 bias=eps_t[0:tp], scale=1.0 / C)
            rstd = pool.tile([128, 1], f32, name="ln_rstd", tag="ln_rstd", bufs=4)
            nc.vector.reciprocal(rstd[0:tp], std[0:tp])
            nc.vector.tensor_scalar_mul(dst_ap, xc[0:tp], rstd[0:tp])

        def late_dma(out_ap, in_ap):
            (nc.gpsimd if POOL_DMA else nc.sync).dma_start(out_ap, in_ap)

        tp_flip = [0]

        def transpose_block(src_full_ap, dst_full_ap):
            """dst[128,128] = src[128,128].T via PE; evictions alternate
            ACT/DVE to balance engine load. Rows beyond the valid token
            count carry garbage into padded dst columns (never read)."""
            ps = psum.tile([128, 128], bf16, name="tps", tag="tp", bufs=2)
            nc.tensor.transpose(ps[:], src_full_ap, ident[:])
            tp_flip[0] ^= 1
            if tp_flip[0]:
                nc.scalar.copy(dst_full_ap, ps[:])
            else:
                nc.vector.tensor_copy(dst_full_ap, ps[:])

        for _rep in range(REPS):
            # ================= attention era =================
            with tc.tile_pool(name="era", bufs=1) as era:
                xres = {}
                xT = {}
                qkT = {}
                vsb = {}
                osb = {}
                x2 = {}

                with tc.tile_pool(name="wqkv", bufs=1) as wp:
                    wqk = wp.tile([128, KC, 2 * C], f8, name="wqk")
                    wv = wp.tile([128, KC, C], f8, name="wv")

                    if _rep == 0:
                        nc.sync.dma_start(bias_pack[:], bp_d[:])
                        nc.sync.dma_start(mb[:],
                                          mb_d[:].rearrange("s p t -> p s t"))

                    # ---- LN1 + transpose to xT; then fold proj bias into
                    # the resident residual (in place, after LN reads) ----
                    for s in range(S):
                        xres[s] = era.tile([128, NT, C], f32, name=f"xres{s}",
                                           tag="xres", bufs=2)
                        xT[s] = era.tile([128, KC, 640], f8, name=f"xT{s}",
                                         tag="xT", bufs=2)
                        for ti, (t0, tp) in enumerate(TOK_TILES):
                            nc.sync.dma_start(xres[s][0:tp, ti, :],
                                              x_d[s, t0:t0 + tp, :])
                            xh = era.tile([128, C], bf16, name="xh", tag="xh",
                                          bufs=3)
                            if tp < 128:
                                nc.vector.memset(xh[96:128, :], 0.0)
                            layer_norm(era, xres[s][0:tp, ti, :], tp, xh[0:tp])
                            for f in range(KC):
                                transpose_block(xh[:, f * 128:(f + 1) * 128],
                                                xT[s][:, f, t0:t0 + 128])
                            nc.gpsimd.tensor_add(xres[s][0:tp, ti, :],
                                                 xres[s][0:tp, ti, :],
                                                 bpj[0:tp, :])

                    nc.sync.dma_start(wqk[:], wqk_d[:])
                    nc.sync.dma_start(wv[:], wv_d[:])
                    # ---- QKV projections (fp8 DoubleRow over K pairs) ----
                    for s in range(S if LVL >= 2 else 0):
                        qkT[s] = era.tile([128, MQK, N], f8, name=f"qkT{s}",
                                          tag="qkT", bufs=2)
                        for m in range(MQK):
                            for (q0, qn) in Q_CHUNKS:
                                ps = ps_tile("ps_qk", qn)
                                for kp in range(KC // 2):
                                    nc.tensor.matmul(
                                        ps[:, 0:qn],
                                        wqk[:, 2 * kp:2 * kp + 2,
                                            m * 128:(m + 1) * 128],
                                        xT[s][:, 2 * kp:2 * kp + 2, q0:q0 + qn],
                                        start=(kp == 0), stop=(kp == 2),
                                        perf_mode=DR)
                                # (ps + S*b) * (1/S)
                                nc.vector.tensor_scalar(
                                    qkT[s][:, m, q0:q0 + qn], ps[:, 0:qn],
                                    qkb[:, m:m + 1], DS_QK, Op.add, Op.mult)
                        vsb[s] = era.tile([128, NT, H * 65], bf16, name=f"v{s}",
                                          tag="v", bufs=2)
                        for ti, (t0, tp) in enumerate(TOK_TILES):
                            for (n0, nn) in C_CHUNKS:
                                ps = ps_tile("ps_v", nn)
                                for kp in range(KC // 2):
                                    nc.tensor.matmul(
                                        ps[0:tp, 0:nn],
                                        xT[s][:, 2 * kp:2 * kp + 2, t0:t0 + tp],
                                        wv[:, 2 * kp:2 * kp + 2, n0:n0 + nn],
                                        start=(kp == 0), stop=(kp == 2),
                                        perf_mode=DR)
                                nh = nn // 64
                                h0 = n0 // 64
                                vview = vsb[s][0:tp, ti, :].rearrange(
                                    "p (h e) -> p h e", e=65)[:, h0:h0 + nh, 0:64]
                                nc.vector.scalar_tensor_tensor(
                                    vview,
                                    ps[0:tp, 0:nn].rearrange("p (h e) -> p h e",
                                                             e=64),
                                    DS_V,
                                    vb[0:tp, n0:n0 + nn].rearrange(
                                        "p (h e) -> p h e", e=64),
                                    Op.mult, Op.add)
                            ones = vsb[s][0:tp, ti, :].rearrange(
                                "p (h e) -> p h e", e=65)[:, :, 64:65]
                            nc.vector.memset(ones, 1.0)

                # ---- attention core (head pairs; score matmuls row-tiled
                # onto complementary PE halves run concurrently) ----
                for s in range(S if LVL >= 3 else 0):
                    osb[s] = era.tile([128, NT, C], bf16, name=f"o{s}",
                                      tag="o", bufs=2)
                    nc.vector.memset(osb[s][96:128, NT - 1, :], 0.0)
                with tc.tile_pool(name="attn", bufs=1) as apool:
                    for s in range(S if LVL >= 3 else 0):
                        for hp in range(H // 2):
                            rpb2 = apool.tile([128, 2, NT, N], f8, name="rpb2",
                                              tag="rpb", bufs=2)
                            nc.sync.dma_start(
                                rpb2[:],
                                rpb_d[2 * hp:2 * hp + 2].rearrange(
                                    "h p t n -> p h t n"))
                            mtile = KC + hp
                            qtile = hp
                            expT = {}
                            for hl in range(2):
                                expT[hl] = apool.tile([128, NT, N], f8,
                                                      name=f"expT{hl}",
                                                      tag="expT", bufs=4)
                            for kt, (k0, tp) in enumerate(TOK_TILES):
                                eraw = {}
                                pss = {}
                                for (q0, qn) in Q_CHUNKS:
                                    for hl in range(2):
                                        base = hl * 64
                                        pss[hl] = ps_tile("ps_sc", qn,
                                                          force_big=True)
                                        nc.tensor.matmul(
                                            pss[hl][0:tp, 0:qn],
                                            qkT[s][base:base + 64, mtile,
                                                   k0:k0 + tp],
                                            qkT[s][base:base + 64, qtile,
                                                   q0:q0 + qn],
                                            start=True, stop=True)
                                    for hl in range(2):
                                        if q0 == 0:
                                            eraw[hl] = apool.tile(
                                                [128, N], bf16, name="eraw",
                                                tag="eraw", bufs=4)
                                        nc.scalar.activation(
                                            eraw[hl][0:tp, q0:q0 + qn],
                                            pss[hl][0:tp, 0:qn], Af.Exp,
                                            bias=mb[0:tp, s, kt:kt + 1],
                                            scale=SCALE)
                                eng = (nc.gpsimd if (POOL_MUL and kt % 3 == 2)
                                       else nc.vector)
                                for hl in range(2):
                                    eng.tensor_mul(expT[hl][0:tp, kt, :],
                                                   eraw[hl][0:tp, :],
                                                   rpb2[0:tp, hl, kt, :])
                            for hl in range(2):
                                h = 2 * hp + hl
                                for qt, (qq0, qp) in enumerate(TOK_TILES):
                                    ops = psum.tile([128, 65], f32, name="ops",
                                                    tag="tiny", bufs=2)
                                    for kt, (k0, tp) in enumerate(TOK_TILES):
                                        nc.tensor.matmul(
                                            ops[0:qp, :],
                                            expT[hl][0:tp, kt, qq0:qq0 + qp],
                                            vsb[s][0:tp, kt,
                                                   h * 65:(h + 1) * 65],
                                            start=(kt == 0),
                                            stop=(kt == NT - 1))
                                    rc = era.tile([128, 1], f32, name="rc",
                                                  tag="rc", bufs=4)
                                    nc.vector.reciprocal(rc[0:qp],
                                                         ops[0:qp, 64:65])
                                    nc.vector.tensor_scalar_mul(
                                        osb[s][0:qp, qt, h * 64:(h + 1) * 64],
                                        ops[0:qp, 0:64], rc[0:qp])

                # ---- proj + residual ----
                with tc.tile_pool(name="proj", bufs=1) as pp:
                    wpj = pp.tile([128, KC, C], f8, name="wpj")
                    nc.sync.dma_start(wpj[:], wpj_d[:])
                    for s in range(S if LVL >= 4 else 0):
                        oT = era.tile([128, KC, 640], f8, name=f"oT{s}",
                                      tag="xT", bufs=2)
                        for ti, (t0, tp) in enumerate(TOK_TILES):
                            for f in range(KC):
                                transpose_block(
                                    osb[s][:, ti, f * 128:(f + 1) * 128],
                                    oT[:, f, t0:t0 + 128])
                        x2[s] = era.tile([128, NT, C], f32, name=f"x2_{s}",
                                         tag="x2", bufs=2)
                        for ti, (t0, tp) in enumerate(TOK_TILES):
                            for (n0, nn) in C_CHUNKS:
                                ps = ps_tile("ps_pj", nn)
                                for kp in range(KC // 2):
                                    nc.tensor.matmul(
                                        ps[0:tp, 0:nn],
                                        oT[:, 2 * kp:2 * kp + 2, t0:t0 + tp],
                                        wpj[:, 2 * kp:2 * kp + 2, n0:n0 + nn],
                                        start=(kp == 0), stop=(kp == 2),
                                        perf_mode=DR)
                                nc.vector.scalar_tensor_tensor(
                                    x2[s][0:tp, ti, n0:n0 + nn],
                                    ps[0:tp, 0:nn], DS_PJ,
                                    xres[s][0:tp, ti, n0:n0 + nn],
                                    Op.mult, Op.add)

                # ---- repack x2 -> text [80, C] + img [1152 (9x128), C] ----
                for s in range(S if LVL >= 4 else 0):
                    nc.sync.dma_start(x2rep_txt[40 * s:40 * s + 40, :],
                                      x2[s][0:40, 0, :])
                    # img rows: seq 40..616 -> global 576*s ..
                    g = 576 * s
                    for kt, (t0, tp) in enumerate(TOK_TILES):
                        p0 = 40 if kt == 0 else 0
                        length = tp - p0
                        src_off = p0
                        while length > 0:
                            j, dp = g // 128, g % 128
                            piece = min(length, 128 - dp)
                            nc.sync.dma_start(
                                x2rep_img[dp:dp + piece, j, :],
                                x2[s][src_off:src_off + piece, kt, :])
                            g += piece
                            src_off += piece
                            length -= piece

            # ================= FFN era =================
            if LVL >= 5:
                with tc.tile_pool(name="ffn", bufs=1) as fp:
                    w1i = fp.tile([128, KC, DFF], f8, name="w1i")
                    w2i = fp.tile([128, KF, C], f8, name="w2i")
                    for k in range(KC):
                        nc.sync.dma_start(w1i[:, k, :], w1i_d[:, k, :])
                    nc.sync.dma_start(w2i[:, 0:12, :], w2i_d[:, 0:12, :])
                    nc.sync.dma_start(w2i[:, 12:24, :], w2i_d[:, 12:24, :])
                    # LN2 + transpose
                    ztT = fp.tile([128, KC, 128], f8, name="ztT")
                    xh2 = fp.tile([128, C], bf16, name="xh2", tag="xh2", bufs=2)
                    nc.vector.memset(xh2[64:128, :], 0.0)
                    layer_norm(fp, x2rep_txt[0:TXTTOK], TXTTOK, xh2[0:TXTTOK])
                    for f in range(KC):
                        transpose_block(xh2[:, f * 128:(f + 1) * 128],
                                        ztT[:, f, 0:128])
                    ziT = fp.tile([128, KC, IMGTOK], f8, name="ziT")
                    for j in range(9):
                        xh2 = fp.tile([128, C], bf16, name="xh2", tag="xh2", bufs=2)
                        layer_norm(fp, x2rep_img[:, j, :], 128, xh2[:])
                        for f in range(KC):
                            transpose_block(xh2[:, f * 128:(f + 1) * 128],
                                            ziT[:, f, j * 128:(j + 1) * 128])
                    # Pre-add the (gamma_2-folded) fc2 biases into the residual so
                    # each fc2 eviction is a single fused op. In-place; Tile
                    # orders these after the LN2 reads above.
                    nc.vector.tensor_add(x2rep_txt[0:TXTTOK, :], x2rep_txt[0:TXTTOK, :],
                                         b2t[0:TXTTOK, :])
                    for j in range(9):
                        nc.vector.tensor_add(x2rep_img[:, j, :], x2rep_img[:, j, :],
                                             b2i[:, :])

                    # ---- img FFN (resident weights, 3 token chunks) ----
                    for c in range(3):
                        q0 = c * IMG_CHUNK
                        hgi = fp.tile([128, KF, IMG_CHUNK], f8, name="hgi",
                                      tag="hgi", bufs=1)
                        for m in range(KF):
                            ps = ps_tile("ps_f1i", 512)
                            for kp in range(KC // 2):
                                nc.tensor.matmul(
                                    ps[:, 0:IMG_CHUNK],
                                    w1i[:, 2 * kp:2 * kp + 2,
                                        m * 128:(m + 1) * 128],
                                    ziT[:, 2 * kp:2 * kp + 2, q0:q0 + IMG_CHUNK],
                                    start=(kp == 0), stop=(kp == 2),
                                    perf_mode=DR)
                            # Gelu(ps * (1/S) + b1) — descale rides the
                            # activation's scale operand.
                            nc.scalar.activation(hgi[:, m, :], ps[:, 0:IMG_CHUNK],
                                                 Af.Gelu, bias=b1i[:, m:m + 1],
                                                 scale=DS_1I)
                        for mt in range(3):
                            j = 3 * c + mt
                            ps0 = ps_tile("ps_f2i0", 512)
                            ps1 = ps_tile("ps_f2i1", 256)
                            for kp in range(KF // 2):
                                nc.tensor.matmul(
                                    ps0[:, 0:512],
                                    hgi[:, 2 * kp:2 * kp + 2,
                                        mt * 128:(mt + 1) * 128],
                                    w2i[:, 2 * kp:2 * kp + 2, 0:512],
                                    start=(kp == 0), stop=(kp == KF // 2 - 1),
                                    perf_mode=DR)
                                nc.tensor.matmul(
                                    ps1[:, 0:256],
                                    hgi[:, 2 * kp:2 * kp + 2,
                                        mt * 128:(mt + 1) * 128],
                                    w2i[:, 2 * kp:2 * kp + 2, 512:768],
                                    start=(kp == 0), stop=(kp == KF // 2 - 1),
                                    perf_mode=DR)
                            ot = fp.tile([128, C], f32, name="ot", tag="ost", bufs=3)
                            for (n0, nn), ps in zip(C_CHUNKS, [ps0, ps1]):
                                nc.vector.scalar_tensor_tensor(
                                    ot[:, n0:n0 + nn], ps[:, 0:nn], DS_2I,
                                    x2rep_img[:, j, n0:n0 + nn],
                                    Op.mult, Op.add)
                            # DMA out: global img row g = 128*j -> (b, 40 + g%576)
                            g0 = 128 * j
                            p = 0
                            while p < 128:
                                g = g0 + p
                                b = g // IMG
                                piece = min(128 - p, IMG * (b + 1) - g)
                                late_dma(
                                    out_d[b, TXT + g - b * IMG:
                                          TXT + g - b * IMG + piece, :],
                                    ot[p:p + piece, :])
                                p += piece

                    # ---- text FFN (streamed weights; fc1 FD=80 stays plain
                    # fp8 — DoubleRow loses below FD 128) ----
                    with tc.tile_pool(name="wtxt", bufs=1) as wt:
                        hgt = fp.tile([128, KF, TXTTOK], f8, name="hgt")
                        for mc in range(8):
                            w1tc = wt.tile([128, 3, KC * 128], f8,
                                           name="w1tc", tag="w1tc", bufs=2)
                            nc.sync.dma_start(
                                w1tc[:],
                                w1t_d[3 * mc:3 * mc + 3].rearrange(
                                    "m p k n -> p m (k n)"))
                            for ml in range(3):
                                m = 3 * mc + ml
                                ps = ps_tile("ps_f1t", 512)
                                for k in range(KC):
                                    nc.tensor.matmul(
                                        ps[:, 0:TXTTOK],
                                        w1tc[:, ml, k * 128:(k + 1) * 128],
                                        ztT[:, k, 0:TXTTOK],
                                        start=(k == 0), stop=(k == KC - 1))
                                nc.scalar.activation(
                                    hgt[:, m, 0:TXTTOK], ps[:, 0:TXTTOK],
                                    Af.Gelu, bias=b1t[:, m:m + 1],
                                    scale=DS_1T)
                        ps0 = ps_tile("ps_f2t0", 512)
                        ps1 = ps_tile("ps_f2t1", 256)
                        for kc4 in range(KF // 4):
                            w2tc = wt.tile([128, 4, C], f8, name="w2tc",
                                           tag="w2tc", bufs=2)
                            nc.sync.dma_start(
                                w2tc[:],
                                w2t_d[4 * kc4:4 * kc4 + 4].rearrange(
                                    "k p n -> p k n"))
                            for kl in range(2):
                                gk = 2 * kc4 + kl            # global pair idx
                                nc.tensor.matmul(
                                    ps0[0:TXTTOK, 0:512],
                                    hgt[:, 4 * kc4 + 2 * kl:
                                        4 * kc4 + 2 * kl + 2, 0:TXTTOK],
                                    w2tc[:, 2 * kl:2 * kl + 2, 0:512],
                                    start=(gk == 0), stop=(gk == KF // 2 - 1),
                                    perf_mode=DR)
                                nc.tensor.matmul(
                                    ps1[0:TXTTOK, 0:256],
                                    hgt[:, 4 * kc4 + 2 * kl:
                                        4 * kc4 + 2 * kl + 2, 0:TXTTOK],
                                    w2tc[:, 2 * kl:2 * kl + 2, 512:768],
                                    start=(gk == 0), stop=(gk == KF // 2 - 1),
                                    perf_mode=DR)
                        ot = fp.tile([128, C], f32, name="ot", tag="ost", bufs=3)
                        for (n0, nn), ps in zip(C_CHUNKS, [ps0, ps1]):
                            nc.vector.scalar_tensor_tensor(
                                ot[0:TXTTOK, n0:n0 + nn],
                                ps[0:TXTTOK, 0:nn], DS_2T,
                                x2rep_txt[0:TXTTOK, n0:n0 + nn],
                                Op.mult, Op.add)
                        for s in range(S):
                            late_dma(out_d[s, 0:TXT, :],
                                     ot[40 * s:40 * s + 40, :])

    nc.compile()
    return nc


_CACHE = {}


def _get_program():
    if "nc" not in _CACHE:
        _CACHE["nc"] = build_program()
    return _CACHE["nc"]


def run(inputs, trace=False):
    from concourse.bass_utils import run_bass_kernel_spmd
    shared, per_core = host_prep(inputs)
    nc = _get_program()
    in_maps = [{**shared, **pc} for pc in per_core]
    res = run_bass_kernel_spmd(nc, in_maps, core_ids=list(range(NCORES)),
                               trace=trace)
    out = np.concatenate([res.results[i]["out"] for i in range(NCORES)],
                         axis=0).astype(np.float32)
    return out, res


def kernel(**inputs):
    out, _ = run(inputs, trace=False)
    return out
